# revision 26
# baseline (speedup 1.0000x reference)
"""CMG MoE-routing kernel for Trainium2 (8 NeuronCores, data-parallel on batch).

Reference computation (per sample b):
  x = concat(motion, command)                      # [B, 576]
  g = elu(x@g_w1+g_b1); g = elu(g@g_w2+g_b2)
  coeffs = softmax(g@g_w3+g_b3)                    # [B, 8]
  for l in 0..5: x = sum_e coeffs[:,e]*(x@W_l[e]+b_l[e]); elu between layers
  out = x                                          # [B, 512]

Device strategy (per core, B_local = 1024):
  - Activations live transposed in SBUF: xT[dim, B] as [128, kt, B] tiles.
    Host pre-transposes/pads/tiles inputs, post-transposes the output.
  - All matmul operands are fp16 (PE runs fp16 at the same 1 col/cycle rate
    as fp32r but LDWEIGHTS streams half the bytes, halving weight-port
    pressure and weight DMA). PSUM accumulation stays fp32; end-to-end
    rel err ~2e-3, well inside the 2e-2 gate.
  - Experts are processed in PAIRS: each PSUM group accumulates both
    experts of a pair (and, for pair 0, the blended-bias matmul
    bias_stack.T @ coeffs). This halves PSUM groups and cuts the
    SBUF y-accumulation traffic vs per-expert groups, while the staged
    scaled activations (xe = coeff_e * x) for one pair fit in 32KB/part.
  - The next layer's pair-0 xe tiles are built interleaved with the
    current layer's last-pair evictions, so the PE crosses layer
    boundaries without draining.
"""
import sys
sys.path.insert(0, "/opt/trn_rl_repo")

import numpy as np

B = 8192
N_CORES = 8
B_LOC = B // N_CORES          # 1024
MOTION = 512
COMMAND = 64
IN_DIM = MOTION + COMMAND     # 576
IN_PAD = 640                  # 5 * 128
HID = 1024
E = 8
NP = 4                        # expert pairs
OUT = 512
P = 128
NCH = 2                       # batch chunks per matmul (N = B_LOC / NCH = 512)
CH = B_LOC // NCH

LAYER_KT = [IN_PAD // P, 8, 8, 8, 8, 8]
LAYER_MT = [8, 8, 8, 8, 8, OUT // P]

_CACHED = None


def _build_program():
    import concourse.tile as tile
    from concourse import mybir, bacc

    f32 = mybir.dt.float32
    f16 = mybir.dt.float16
    ACT = mybir.ActivationFunctionType
    ALU = mybir.AluOpType

    nc = bacc.Bacc("TRN2", target_bir_lowering=False, debug=False)

    # ---- DRAM I/O (host-pre-tiled; every DMA contiguous) -------------------
    kt0 = IN_PAD // P
    xt_d = nc.dram_tensor("xt", [P, kt0, B_LOC], f16, kind="ExternalInput")
    gw1_d = nc.dram_tensor("gw1", [HID // P, P, kt0, P], f16, kind="ExternalInput")
    gw2_d = nc.dram_tensor("gw2", [HID // P, P, HID // P, P], f16, kind="ExternalInput")
    gw3_d = nc.dram_tensor("gw3", [P, HID // P, E], f16, kind="ExternalInput")
    gb1_d = nc.dram_tensor("gb1", [P, HID // P], f32, kind="ExternalInput")
    gb2_d = nc.dram_tensor("gb2", [P, HID // P], f32, kind="ExternalInput")
    gb3_d = nc.dram_tensor("gb3", [E, 1], f32, kind="ExternalInput")
    w_d, b_d = [], []
    for l in range(6):
        kt, mt = LAYER_KT[l], LAYER_MT[l]
        # per (pair, m) tile: [P, kt, 2, P]
        w_d.append(nc.dram_tensor(f"w{l}", [NP, mt, P, kt, 2, P], f16,
                                  kind="ExternalInput"))
        b_d.append(nc.dram_tensor(f"b{l}", [E, mt * P], f16, kind="ExternalInput"))
    basis_d = nc.dram_tensor("basis", [E, E, P], f16, kind="ExternalInput")
    ones_d = nc.dram_tensor("ones", [E, E], f16, kind="ExternalInput")
    out_d = nc.dram_tensor("out", [P, OUT // P, B_LOC], f32, kind="ExternalOutput")

    with tile.TileContext(nc) as tc:
        with tc.tile_pool(name="xtp", bufs=1) as xtp, \
             tc.tile_pool(name="xp", bufs=2) as xp, \
             tc.tile_pool(name="xe", bufs=2) as xe_pool, \
             tc.tile_pool(name="yp", bufs=1) as yp, \
             tc.tile_pool(name="cp", bufs=1) as cp, \
             tc.tile_pool(name="wt", bufs=4) as wt_pool, \
             tc.tile_pool(name="gwt", bufs=3) as gwt_pool, \
             tc.tile_pool(name="sm", bufs=1) as sm, \
             tc.tile_pool(name="bt", bufs=2) as bt_pool, \
             tc.tile_pool(name="et", bufs=2) as et, \
             tc.tile_pool(name="ps", bufs=3, space="PSUM") as ps, \
             tc.tile_pool(name="ps2", bufs=1, space="PSUM") as ps2:

            # ---- input activations (k=0 first so the PE starts sooner) ----
            xt = xtp.tile([P, kt0, B_LOC], f16, tag="xt")
            nc.sync.dma_start(xt[:, 0:1, :], xt_d.ap()[:, 0:1, :])

            # xe pair buffers; gating g1/g2 alias slots of the first two
            xeA = xe_pool.tile([P, 2, 8, B_LOC], f16, tag="xe")
            xeB = xe_pool.tile([P, 2, 8, B_LOC], f16, tag="xe")
            g1 = xeA[:, 0, :, :]
            g2 = xeA[:, 1, :, :]

            def elu1_evict_bias(psum, bias_col, nbias_col, dst_ap):
                """dst = elu(psum + bias) + 1 ; bias per-partition [P,1].
                Chunked in halves; work split across Scalar and DVE so
                neither engine paces the gating phase."""
                r = et.tile([P, B_LOC], f16, tag="elu_r")
                u = et.tile([P, B_LOC], f16, tag="elu_u")
                r2 = et.tile([P, B_LOC], f16, tag="elu_r2")
                for c in range(NCH):
                    s = slice(c * CH, (c + 1) * CH)
                    nc.scalar.activation(r[:, s], psum[:, s], ACT.Relu,
                                         scale=-1.0, bias=nbias_col)
                    nc.scalar.activation(u[:, s], r[:, s], ACT.Exp, scale=-1.0)
                    nc.vector.tensor_scalar(r2[:, s], psum[:, s], bias_col, 0.0,
                                            ALU.add, ALU.max)
                    nc.vector.tensor_tensor(dst_ap[:, s], u[:, s], r2[:, s], ALU.add)

            # ---- gating network (all fp16 operands) -----------------------
            def dense_layer(w_dram, bias_dram, kt, rhs3, out_tile):
                bias_sb = et.tile([P, 8], f32, tag="gbias")
                nbias_sb = et.tile([P, 8], f32, tag="gnbias")
                wt0 = gwt_pool.tile([P, 8, P], f16, tag="gwt")
                nc.sync.dma_start(wt0[:, :kt, :], w_dram.ap()[0])
                if w_dram is gw1_d:
                    for k in range(1, kt0):
                        nc.sync.dma_start(xt[:, k:k + 1, :], xt_d.ap()[:, k:k + 1, :])
                nc.sync.dma_start(bias_sb[:], bias_dram.ap())
                nc.vector.tensor_scalar(nbias_sb[:], bias_sb[:], -1.0, None, ALU.mult)
                for m in range(HID // P):
                    if m == 0:
                        wt = wt0
                    else:
                        wt = gwt_pool.tile([P, 8, P], f16, tag="gwt")
                        nc.sync.dma_start(wt[:, :kt, :], w_dram.ap()[m])
                    psum = ps.tile([P, B_LOC], f32, tag="ps")
                    for k in range(kt):
                        for c in range(NCH):
                            s = slice(c * CH, (c + 1) * CH)
                            nc.tensor.matmul(psum[:, s], wt[:, k, :], rhs3[:, k, s],
                                             start=(k == 0), stop=(k == kt - 1))
                    elu1_evict_bias(psum, bias_sb[:, m:m + 1], nbias_sb[:, m:m + 1],
                                    out_tile[:, m, :])

            dense_layer(gw1_d, gb1_d, kt0, xt, g1)

            # small softmax constants: DMA'd off the critical chain
            gw3_sb = sm.tile([P, 8, E], f16, tag="gw3")
            nc.sync.dma_start(gw3_sb[:], gw3_d.ap())
            gb3_sb = sm.tile([E, 1], f32, tag="gb3")
            nc.sync.dma_start(gb3_sb[:], gb3_d.ap())
            ones_sb = sm.tile([E, E], f16, tag="ones")
            nc.sync.dma_start(ones_sb[:], ones_d.ap())
            basis = sm.tile([E, E, P], f16, tag="basis")
            nc.sync.dma_start(basis[:], basis_d.ap())

            dense_layer(gw2_d, gb2_d, HID // P, g1, g2)

            # logits: [E, B] = gw3.T @ g2
            ps_log = ps2.tile([P, B_LOC], f32, tag="ps2")
            for k in range(HID // P):
                for c in range(NCH):
                    s = slice(c * CH, (c + 1) * CH)
                    nc.tensor.matmul(ps_log[:E, s], gw3_sb[:, k, :], g2[:, k, s],
                                     start=(k == 0), stop=(k == HID // P - 1))

            # softmax over partitions 0..7 (no Ln: stay on one ACT table)
            ex = et.tile([E, B_LOC], f16, tag="elu_r")
            nc.scalar.activation(ex[:], ps_log[:E, :], ACT.Exp, bias=gb3_sb[:])
            ones8 = ones_sb[:, 0:1]
            ps_den = ps2.tile([P, B_LOC], f32, tag="ps2")
            for c in range(NCH):
                s = slice(c * CH, (c + 1) * CH)
                nc.tensor.matmul(ps_den[:1, s], ones8, ex[:, s], start=True, stop=True)
            # 1/den on DVE (custom op, ~51 ULP), f16 copy, broadcast matmul
            recip = et.tile([1, B_LOC], f32, tag="elu_v")
            nc.vector.reciprocal_approx_fast(out=recip[:], in_=ps_den[:1, :])
            recip16 = et.tile([1, B_LOC], f16, tag="elu_u")
            nc.scalar.activation(recip16[:], recip[:], ACT.Copy)
            ones1x8 = ones_sb[0:1, :]
            ps_rb = ps2.tile([P, B_LOC], f32, tag="ps2")
            for c in range(NCH):
                s = slice(c * CH, (c + 1) * CH)
                nc.tensor.matmul(ps_rb[:E, s], ones1x8, recip16[:, s],
                                 start=True, stop=True)
            coeffs = sm.tile([E, B_LOC], f16, tag="coeffs")
            nc.vector.tensor_tensor(coeffs[:], ex[:], ps_rb[:E, :], ALU.mult)

            # replicate each coeff row across 128 partitions: cmat[:, e, :]
            # (ps pool, bufs=3, so broadcast e+1 overlaps the copy of e)
            cmat = cp.tile([P, E, B_LOC], f16, tag="C")
            for e in range(E):
                ps_c = ps.tile([P, B_LOC], f32, tag="ps")
                for c in range(NCH):
                    s = slice(c * CH, (c + 1) * CH)
                    nc.tensor.matmul(ps_c[:, s], basis[:, e, :], coeffs[:, s],
                                     start=True, stop=True)
                nc.scalar.activation(cmat[:, e, :], ps_c[:], ACT.Copy)

            # ---- MoE stack -------------------------------------------------
            y = yp.tile([P, 8, B_LOC], f32, tag="y")

            def build_xe_slab(xe_t, src3, k, ep):
                """xe_t[:, 0/1, k, :] = src3[:, k, :] * cmat[:, 2ep+eo, :]"""
                for eo in range(2):
                    nc.vector.tensor_tensor(xe_t[:, eo, k, :], src3[:, k, :],
                                            cmat[:, 2 * ep + eo, :], ALU.mult)

            def evict_elu(y_ap, dst_ap):
                """dst = elu(y_ap); y fp32 [P, B_LOC] SBUF -> fp16."""
                r = et.tile([P, B_LOC], f16, tag="elu_r")
                u = et.tile([P, B_LOC], f16, tag="elu_u")
                v = et.tile([P, B_LOC], f16, tag="elu_v")
                nc.scalar.activation(r[:], y_ap, ACT.Relu, scale=-1.0)
                nc.scalar.activation(u[:], r[:], ACT.Exp, scale=-1.0)
                nc.vector.tensor_scalar(v[:], y_ap, 0.0, 1.0, ALU.max, ALU.subtract)
                nc.vector.tensor_tensor(dst_ap, u[:], v[:], ALU.add)

            cur = xt
            xe_bufs = [xeA, xeB]
            # layer-0 pair-0 xe (after cmat)
            for k in range(LAYER_KT[0]):
                build_xe_slab(xeA, cur, k, 0)

            for l in range(6):
                kt, mt = LAYER_KT[l], LAYER_MT[l]
                bst = bt_pool.tile([E, 8 * P], f16, tag="bst")
                nc.sync.dma_start(bst[:, :mt * P], b_d[l].ap())
                if l < 5:
                    nxt = xp.tile([P, 8, B_LOC], f16, tag="xt")
                for ep in range(NP):
                    xe = xe_bufs[ep % 2]
                    for m in range(mt):
                        wt = wt_pool.tile([P, 8, 2, P], f16, tag="wt")
                        nc.sync.dma_start(wt[:, :kt, :, :], w_d[l].ap()[ep, m])
                        psum = ps.tile([P, B_LOC], f32, tag="ps")
                        for k in range(kt):
                            for eo in range(2):
                                for c in range(NCH):
                                    s = slice(c * CH, (c + 1) * CH)
                                    nc.tensor.matmul(
                                        psum[:, s], wt[:, k, eo, :], xe[:, eo, k, s],
                                        start=(k == 0 and eo == 0),
                                        stop=(k == kt - 1 and eo == 1 and ep != 0),
                                    )
                        if ep == 0:
                            # blended-bias matmul closes the group
                            for c in range(NCH):
                                s = slice(c * CH, (c + 1) * CH)
                                nc.tensor.matmul(psum[:, s],
                                                 bst[:, m * P:(m + 1) * P],
                                                 coeffs[:, s], start=False, stop=True)
                            nc.scalar.activation(y[:, m, :], psum[:], ACT.Copy)
                        elif not (l == 5 and ep == NP - 1):
                            nc.vector.tensor_tensor(y[:, m, :], psum[:], y[:, m, :],
                                                    ALU.add)
                        else:
                            # final layer: chunked accumulate + store
                            for c in range(NCH):
                                s = slice(c * CH, (c + 1) * CH)
                                nc.vector.tensor_tensor(y[:, m, s], psum[:, s],
                                                        y[:, m, s], ALU.add)
                                nc.sync.dma_start(out_d.ap()[:, m, s], y[:, m, s])
                        if ep < NP - 1:
                            # build the NEXT pair's xe slabs, spread across
                            # this pair's m-steps so the TTs sit between the
                            # y-ops in the in-order DVE queue (building them
                            # in one block would delay them past the last
                            # y-add and stall the next pair's first groups)
                            for k in range(m * kt // mt, (m + 1) * kt // mt):
                                build_xe_slab(xe_bufs[(ep + 1) % 2], cur, k, ep + 1)
                        elif l < 5:
                            # finished m-tile: evict + build next layer's
                            # pair-0 xe slab in the same breath
                            evict_elu(y[:, m, :], nxt[:, m, :])
                            if m < LAYER_KT[l + 1]:
                                build_xe_slab(xe_bufs[0], nxt, m, 0)
                if l < 5:
                    cur = nxt

    nc.compile()
    return nc


def _prep_gw(w, pad_to=None):
    """[din, dout] -> [mt, P, kt, P] fp16 contiguous lhsT tiles (din padded)."""
    din, dout = w.shape
    if pad_to is not None and pad_to != din:
        wp = np.zeros((pad_to, dout), np.float32)
        wp[:din] = w
        w, din = wp, pad_to
    kt, mt = din // P, dout // P
    return np.ascontiguousarray(
        w.reshape(kt, P, mt, P).transpose(2, 1, 0, 3)).astype(np.float16)


def _prep_we(w, pad_to=None):
    """[E, din, dout] -> [NP, mt, P, kt, 2, P] fp16 (pair-packed lhsT tiles)."""
    e, din, dout = w.shape
    if pad_to is not None and pad_to != din:
        wp = np.zeros((e, pad_to, dout), np.float32)
        wp[:, :din] = w
        w, din = wp, pad_to
    kt, mt = din // P, dout // P
    # [E, kt, P, mt, P] -> [NP, 2, kt, P, mt, P] -> [NP, mt, P, kt, 2, P]
    t = w.reshape(NP, 2, kt, P, mt, P).transpose(0, 4, 3, 2, 1, 5)
    return np.ascontiguousarray(t).astype(np.float16)


def _make_in_maps(inputs):
    motion = np.asarray(inputs["motion"], np.float32)
    command = np.asarray(inputs["command"], np.float32)

    gw2 = np.asarray(inputs["g_w2"], np.float32)
    gw3 = np.asarray(inputs["g_w3"], np.float32)
    gw3_f16 = gw3.astype(np.float16)
    gw2_f16c = gw2.astype(np.float16).astype(np.float32)
    shared = {
        "gw1": _prep_gw(np.asarray(inputs["g_w1"], np.float32), pad_to=IN_PAD),
        "gw2": _prep_gw(gw2),
        "gw3": np.ascontiguousarray(
            gw3.reshape(HID // P, P, E).transpose(1, 0, 2)).astype(np.float16),
        # gating activations carry elu(z)+1; fold the -1 into next biases
        # (colsums taken over the fp16-quantized weights actually used)
        "gb1": np.ascontiguousarray(
            np.asarray(inputs["g_b1"], np.float32).reshape(HID // P, P).T),
        "gb2": np.ascontiguousarray(
            (np.asarray(inputs["g_b2"], np.float32) - gw2_f16c.sum(0))
            .reshape(HID // P, P).T),
        "gb3": np.ascontiguousarray(
            (np.asarray(inputs["g_b3"], np.float32)
             - gw3_f16.astype(np.float32).sum(0)).reshape(E, 1)),
    }
    for l in range(6):
        w = np.asarray(inputs[f"w{l}"], np.float32)
        bias = np.asarray(inputs[f"b{l}"], np.float32)
        shared[f"w{l}"] = _prep_we(w, pad_to=IN_PAD if l == 0 else None)
        shared[f"b{l}"] = np.ascontiguousarray(bias).astype(np.float16)

    basis_np = np.zeros((E, E, P), np.float16)
    for e in range(E):
        basis_np[e, e, :] = 1.0
    shared["basis"] = basis_np
    shared["ones"] = np.ones((E, E), np.float16)

    x_cat = np.concatenate([motion, command], axis=1)
    x_pad = np.zeros((B, IN_PAD), np.float32)
    x_pad[:, :IN_DIM] = x_cat
    in_maps = []
    for c in range(N_CORES):
        xs = x_pad[c * B_LOC:(c + 1) * B_LOC]
        xt = np.ascontiguousarray(
            xs.T.reshape(IN_PAD // P, P, B_LOC).transpose(1, 0, 2)).astype(np.float16)
        in_maps.append({"xt": xt, **shared})
    return in_maps


def _assemble_out(core_outs):
    outs = []
    for o in core_outs:                                    # [P, OUT/P, B_LOC]
        outs.append(o.transpose(2, 1, 0).reshape(B_LOC, OUT))
    return np.concatenate(outs, axis=0).astype(np.float32)


def kernel(**inputs):
    global _CACHED
    from concourse import bass_utils

    if _CACHED is None:
        _CACHED = _build_program()
    nc = _CACHED

    in_maps = _make_in_maps(inputs)
    res = bass_utils.run_bass_kernel_spmd(
        nc, in_maps, core_ids=list(range(N_CORES)), trace=False)
    return _assemble_out([res.results[c]["out"] for c in range(N_CORES)])
